# revision 3
# baseline (speedup 1.0000x reference)
"""KAN layer kernel for trn2 (8 NeuronCores, SPMD data-parallel over tokens).

Math: reference computes, per element x with t = tanh(x):
  out[n,o] = sum_i W[o,i] * (c0_i*B0(t_ni) + c1_i*B1(t_ni))
where B0/B1 are cubic B-splines on knots linspace(-1,1,8) (n_active = 2).

Closed form: 6*B_k(t) = E_k(t) = p^3 - 4 q^3 with
  p = relu(2 - |3.5 t + b_k|), q = relu(1 - |3.5 t + b_k|), b_0=1.5, b_1=0.5.
Each E_k is computed with TWO custom DVE passes of the shared shape
  out = [Src1 +] (C0 * relu(C1 - |Src0 - C2|))^3
using ABSOLUTE_DIFF to fold the abs into one ALU op:
  p-pass: C0 = 3.5,          C1 = 4/7, C2 = -b/3.5  ->  p^3
  q-pass: C0 = 3.5*cbrt(-4), C1 = 2/7, C2 = -b/3.5  ->  -4 q^3 (accumulated)
The per-channel combine and 1/6 fold into the output matmul weights:
  out = E0 @ M0T + E1 @ M1T,  MkT[i,o] = W[o,i] * c_k[i] / 6.

Schedule (per core, 8 chunks of 1024 tokens):
  produce(c): DMA x fp32 -> PE transpose raw x (per 512-tok sub-tile, PSUM)
              -> ACT tanh PSUM->SBUF fp16 (packed [128, 2*chunk])
  consume(c): 4 DVE passes fp16 -> PE matmuls fp16 -> ACT copy -> DMA out fp16
Emission is software-pipelined (produce(c+1) before consume(c)) so the PE
transposes for the next chunk sit ahead of the current chunk's matmuls in PE
program order and the DVE (critical engine, ~70us) never starves.
Output is fp16 on device, upcast to fp32 on host (tolerance 2e-2 >> fp16 eps).
"""

import sys

sys.path.insert(0, "/opt/trn_rl_repo")

import numpy as np

CP = -(4.0 ** (1.0 / 3.0))  # cbrt(-4): (CP*q)^3 = -4 q^3

N_CORES = 8
TOK_TOTAL = 16 * 4096
TOK_PER_CORE = TOK_TOTAL // N_CORES  # 8192
IN_DIM = 256
OUT_DIM = 256

_CACHE = {}


def _register_ops():
    from concourse import dve_ops
    from concourse.dve_ops import DveOp, OPS, CUSTOM_DVE_SPECS
    from concourse.dve_spec import (
        Spec, Src0, Src1, C0, C1, C2, relu, sq, lower, Bin, _has_src1,
    )
    from concourse.dve_uop import DveOpSpec, AluOp

    def make(name, spec):
        if name in dve_ops._SUB_OPCODE_FOR_NAME:
            return next(op for op in OPS if op.name == name)
        row = dve_ops._CUSTOM_DVE_ROW_BASE + len(OPS)
        assert row < 0x20
        dve_ops._SUB_OPCODE_FOR_NAME[name] = row
        shas = {}
        for ver in ("v3", "v4"):
            tmp = DveOpSpec(
                name=name, opcode=row, uops=lower(spec, ver=ver),
                rd1_en=_has_src1(spec),
            )
            shas[ver] = tmp.sha(ver)
        op = DveOp(name, spec, subdim=False, uops_sha=shas)
        OPS.append(op)
        CUSTOM_DVE_SPECS[name] = spec
        return op

    # out = (C0 * relu(C1 - |Src0 - C2|))^3, optionally + Src1
    ad = Bin(AluOp.ABSOLUTE_DIFF, Src0, C2)
    rc = relu(C1 - ad) * C0
    cube = rc * sq(rc)

    def cube_ref(in0, in1, s0, s1, imm2):
        r = np.maximum(s1 - np.abs(in0 - imm2), 0.0)
        rcv = (r * s0).astype(np.float32)
        return (rcv * np.square(rcv)).astype(np.float32)

    def cube_acc_ref(in0, in1, s0, s1, imm2):
        r = np.maximum(s1 - np.abs(in0 - imm2), 0.0)
        rcv = (r * s0).astype(np.float32)
        return (in1 + rcv * np.square(rcv)).astype(np.float32)

    KAN_CUBE = make("KAN_CUBE3", Spec(body=cube, reference=cube_ref))
    KAN_CUBE_ACC = make(
        "KAN_CUBE3A", Spec(body=Src1 + cube, reference=cube_acc_ref)
    )
    return KAN_CUBE, KAN_CUBE_ACC


def _build_bass(chunk_tok=1024, sub_tok=512):
    import concourse.bass as bass
    import concourse.bacc as bacc
    import concourse.mybir as mybir
    from concourse import tile

    blocks = chunk_tok // 128          # matmul 128-token blocks per chunk
    subs = chunk_tok // sub_tok        # transpose/tanh sub-tiles per chunk
    sblocks = sub_tok // 128
    chunks = TOK_PER_CORE // chunk_tok

    KAN_CUBE, KAN_CUBE_ACC = _register_ops()

    f32 = mybir.dt.float32
    f16 = mybir.dt.float16
    nc = bacc.Bacc(None, target_bir_lowering=False)

    xs = nc.dram_tensor("xs", [TOK_PER_CORE, IN_DIM], f32, kind="ExternalInput")
    m0t = nc.dram_tensor("m0t", [IN_DIM, OUT_DIM], f16, kind="ExternalInput")
    m1t = nc.dram_tensor("m1t", [IN_DIM, OUT_DIM], f16, kind="ExternalInput")
    ident = nc.dram_tensor("ident", [128, 128], f32, kind="ExternalInput")
    out = nc.dram_tensor("out", [TOK_PER_CORE, OUT_DIM], f16, kind="ExternalOutput")

    Tanh = mybir.ActivationFunctionType.Tanh

    # DVE pass constants: C2 = -b/3.5 per basis; p-pass C1=4/7, q-pass C1=2/7
    C2S = (-1.5 / 3.5, -0.5 / 3.5)
    C1P, C1Q = 4.0 / 7.0, 2.0 / 7.0
    S_P, S_Q = 3.5, 3.5 * CP

    with tile.TileContext(nc) as tc:
        with (
            tc.tile_pool(name="const", bufs=1) as cpool,
            tc.tile_pool(name="sbuf", bufs=2) as pool,
            tc.tile_pool(name="psx", bufs=2, space="PSUM") as ppx,
            tc.tile_pool(name="psa", bufs=1, space="PSUM") as ppa,
        ):
            idt = cpool.tile([128, 128], f32)
            nc.sync.dma_start(idt[:], ident[:])
            # weight halves: wt[k][h] = MkT[h*128:(h+1)*128, :]  (fp16)
            wt = []
            for k, mt in enumerate((m0t, m1t)):
                row = []
                for h in range(2):
                    w = cpool.tile([128, OUT_DIM], f16, tag=f"w{k}{h}")
                    nc.sync.dma_start(w[:], mt[h * 128:(h + 1) * 128, :])
                    row.append(w)
                wt.append(row)

            def produce(c):
                xv = xs[c * chunk_tok:(c + 1) * chunk_tok, :].rearrange(
                    "(a p) i -> p a i", p=128
                )
                xt = pool.tile([128, blocks * IN_DIM], f32, tag="xt")
                nc.sync.dma_start(
                    xt[:].rearrange("p (a i) -> p a i", i=IN_DIM), xv
                )
                # t16 packed [128, 2*chunk]: col = h*chunk + tok
                t16 = pool.tile([128, 2 * chunk_tok], f16, tag="t16")
                t16v = t16[:].rearrange("p (h t) -> p h t", h=2)
                for s in range(subs):
                    xtp = ppx.tile([128, 2 * sub_tok], f32, tag="xtp")
                    for al in range(sblocks):
                        a = s * sblocks + al
                        for h in range(2):
                            nc.tensor.transpose(
                                xtp[:, h * sub_tok + al * 128:
                                    h * sub_tok + (al + 1) * 128],
                                xt[:, a * IN_DIM + h * 128:
                                   a * IN_DIM + (h + 1) * 128],
                                idt[:],
                            )
                    # tanh fused with PSUM->SBUF, downcast to fp16
                    nc.scalar.activation(
                        t16v[:, :, s * sub_tok:(s + 1) * sub_tok],
                        xtp[:].rearrange("p (h t) -> p h t", h=2),
                        Tanh,
                    )
                return t16

            def consume(c, t16):
                es = []
                for k in range(2):
                    r = pool.tile([128, 2 * chunk_tok], f16, tag="r")
                    nc.vector._custom_dve(
                        KAN_CUBE, out=r[:], in0=t16[:],
                        s0=S_P, s1=C1P, imm2=C2S[k],
                    )
                    e = pool.tile([128, 2 * chunk_tok], f16, tag=f"e{k}")
                    nc.vector._custom_dve(
                        KAN_CUBE_ACC, out=e[:], in0=t16[:], in1=r[:],
                        s0=S_Q, s1=C1Q, imm2=C2S[k],
                    )
                    es.append(e)

                acc = ppa.tile([128, blocks * OUT_DIM], f32, tag="acc")
                os_t = pool.tile([128, blocks * OUT_DIM], f16, tag="os")
                for a in range(blocks):
                    for j in range(4):
                        k, h = j // 2, j % 2
                        nc.tensor.matmul(
                            acc[:, a * OUT_DIM:(a + 1) * OUT_DIM],
                            es[k][:, h * chunk_tok + a * 128:
                                  h * chunk_tok + (a + 1) * 128],
                            wt[k][h][:],
                            start=(j == 0),
                            stop=(j == 3),
                        )
                nc.scalar.copy(os_t[:], acc[:])
                ov = out[c * chunk_tok:(c + 1) * chunk_tok, :].rearrange(
                    "(a p) o -> p a o", p=128
                )
                nc.sync.dma_start(
                    ov, os_t[:].rearrange("p (a o) -> p a o", o=OUT_DIM)
                )

            t_cur = produce(0)
            for c in range(chunks):
                t_next = produce(c + 1) if c + 1 < chunks else None
                consume(c, t_cur)
                t_cur = t_next

    nc.compile()
    return nc


CHUNK_TOK_RT = 1024
SUB_TOK_RT = 512


def _get_nc():
    if "nc" not in _CACHE:
        _CACHE["nc"] = _build_bass(CHUNK_TOK_RT, SUB_TOK_RT)
    return _CACHE["nc"]


def kernel(x, inner_coeffs, outer_coeffs):
    from concourse import bass_utils

    x = np.asarray(x, dtype=np.float32)
    inner = np.asarray(inner_coeffs, dtype=np.float32)
    outer = np.asarray(outer_coeffs, dtype=np.float32)

    B, S, I = x.shape
    xf = np.ascontiguousarray(x.reshape(B * S, I))

    # MkT[i,o] = W[o,i] * c_k[i] / 6  (fp16 on device)
    m0 = np.ascontiguousarray((outer.T * inner[:, 0:1]) / 6.0).astype(np.float16)
    m1 = np.ascontiguousarray((outer.T * inner[:, 1:2]) / 6.0).astype(np.float16)
    ident = np.eye(128, dtype=np.float32)

    nc = _get_nc()
    in_maps = []
    for i in range(N_CORES):
        in_maps.append({
            "xs": xf[i * TOK_PER_CORE:(i + 1) * TOK_PER_CORE],
            "m0t": m0, "m1t": m1, "ident": ident,
        })
    res = bass_utils.run_bass_kernel_spmd(nc, in_maps, list(range(N_CORES)))
    outs = [res.results[i]["out"] for i in range(N_CORES)]
    full = np.concatenate(outs, axis=0).astype(np.float32).reshape(B, S, OUT_DIM)
    return full


# revision 7
# speedup vs baseline: 1.1359x; 1.1359x over previous
"""KAN layer kernel for trn2 (8 NeuronCores, SPMD data-parallel over tokens).

Math: reference computes, per element x with t = tanh(x):
  out[n,o] = sum_i W[o,i] * (c0_i*B0(t_ni) + c1_i*B1(t_ni))
where B0/B1 are cubic B-splines on knots linspace(-1,1,8) (n_active = 2).

Closed form: 6*B_k(t) = E_k(t) = p^3 - 4 q^3 with
  p = relu(2 - |3.5 t + b_k|), q = relu(1 - |3.5 t + b_k|), b_0=1.5, b_1=0.5.
Each E_k takes TWO custom DVE passes of the shared shape
  out = [Src1 +] (C0 * relu(C1 - |Src0 - C2|))^3
(ABSOLUTE_DIFF folds the abs into one ALU op; the 8-ALU-stage DVE pipeline
fits exactly one full cube term per pass, so 4 passes/core is minimal):
  p-pass: C0 = 3.5,          C1 = 4/7, C2 = -b/3.5  ->  p^3
  q-pass: C0 = 3.5*cbrt(-4), C1 = 2/7, C2 = -b/3.5  ->  -4 q^3 (accumulated)
The per-channel combine and 1/6 fold into the output matmul weights:
  out = E0 @ M0T + E1 @ M1T,  MkT[i,o] = W[o,i] * c_k[i] / 6.

Per-core schedule: token chunks with ramped sizes (128,128,256,512,...,512,
256,128,128) to shrink pipeline fill and drain. Per chunk:
  DMA x fp32 -> PE transpose raw x (fp32 identity matmul, per <=512-token
  sub-tile into PSUM) -> ACT tanh PSUM->SBUF fp16 -> 4 DVE cube passes fp16
  (the critical engine: ~78us busy of ~85us total) -> PE matmuls fp16 into
  PSUM fp32 -> ACT copy to fp16 -> DMA out.
Output is fp16 on device, upcast to fp32 on host (tolerance 2e-2 >> fp16 eps).
"""

import sys

sys.path.insert(0, "/opt/trn_rl_repo")

import numpy as np

CP = -(4.0 ** (1.0 / 3.0))  # cbrt(-4): (CP*q)^3 = -4 q^3

N_CORES = 8
TOK_TOTAL = 16 * 4096
TOK_PER_CORE = TOK_TOTAL // N_CORES  # 8192
IN_DIM = 256
OUT_DIM = 256

_CACHE = {}


def _register_ops():
    from concourse import dve_ops
    from concourse.dve_ops import DveOp, OPS, CUSTOM_DVE_SPECS
    from concourse.dve_spec import (
        Spec, Src0, Src1, C0, C1, C2, relu, sq, lower, Bin, _has_src1,
    )
    from concourse.dve_uop import DveOpSpec, AluOp

    def make(name, spec):
        if name in dve_ops._SUB_OPCODE_FOR_NAME:
            return next(op for op in OPS if op.name == name)
        row = dve_ops._CUSTOM_DVE_ROW_BASE + len(OPS)
        assert row < 0x20
        dve_ops._SUB_OPCODE_FOR_NAME[name] = row
        shas = {}
        for ver in ("v3", "v4"):
            tmp = DveOpSpec(
                name=name, opcode=row, uops=lower(spec, ver=ver),
                rd1_en=_has_src1(spec),
            )
            shas[ver] = tmp.sha(ver)
        op = DveOp(name, spec, subdim=False, uops_sha=shas)
        OPS.append(op)
        CUSTOM_DVE_SPECS[name] = spec
        return op

    # out = (C0 * relu(C1 - |Src0 - C2|))^3, optionally + Src1
    ad = Bin(AluOp.ABSOLUTE_DIFF, Src0, C2)
    rc = relu(C1 - ad) * C0
    cube = rc * sq(rc)

    def cube_ref(in0, in1, s0, s1, imm2):
        r = np.maximum(s1 - np.abs(in0 - imm2), 0.0)
        rcv = (r * s0).astype(np.float32)
        return (rcv * np.square(rcv)).astype(np.float32)

    def cube_acc_ref(in0, in1, s0, s1, imm2):
        r = np.maximum(s1 - np.abs(in0 - imm2), 0.0)
        rcv = (r * s0).astype(np.float32)
        return (in1 + rcv * np.square(rcv)).astype(np.float32)

    KAN_CUBE = make("KAN_CUBE3", Spec(body=cube, reference=cube_ref))
    KAN_CUBE_ACC = make(
        "KAN_CUBE3A", Spec(body=Src1 + cube, reference=cube_acc_ref)
    )
    return KAN_CUBE, KAN_CUBE_ACC


def _chunk_schedule():
    # ramp up and down to shrink pipeline fill/drain on the DVE stream
    sched = [128, 128, 256, 512]
    tail = [256, 128, 128]
    mid = TOK_PER_CORE - sum(sched) - sum(tail)
    return sched + [512] * (mid // 512) + tail


def _build_bass():
    import concourse.bass as bass
    import concourse.bacc as bacc
    import concourse.mybir as mybir
    from concourse import tile

    KAN_CUBE, KAN_CUBE_ACC = _register_ops()

    f32 = mybir.dt.float32
    f16 = mybir.dt.float16
    nc = bacc.Bacc(None, target_bir_lowering=False)

    xs = nc.dram_tensor("xs", [TOK_PER_CORE, IN_DIM], f32, kind="ExternalInput")
    m0t = nc.dram_tensor("m0t", [IN_DIM, OUT_DIM], f16, kind="ExternalInput")
    m1t = nc.dram_tensor("m1t", [IN_DIM, OUT_DIM], f16, kind="ExternalInput")
    ident = nc.dram_tensor("ident", [128, 128], f32, kind="ExternalInput")
    out = nc.dram_tensor("out", [TOK_PER_CORE, OUT_DIM], f16, kind="ExternalOutput")

    Tanh = mybir.ActivationFunctionType.Tanh

    # DVE pass constants: C2 = -b/3.5 per basis; p-pass C1=4/7, q-pass C1=2/7
    C2S = (-1.5 / 3.5, -0.5 / 3.5)
    C1P, C1Q = 4.0 / 7.0, 2.0 / 7.0
    S_P, S_Q = 3.5, 3.5 * CP

    sched = _chunk_schedule()
    assert sum(sched) == TOK_PER_CORE
    MAXC = max(sched)          # max chunk tokens (fixed tile sizes)
    MAXB = MAXC // 128

    with tile.TileContext(nc) as tc:
        with (
            tc.tile_pool(name="const", bufs=1) as cpool,
            tc.tile_pool(name="sbuf", bufs=4) as pool,
            tc.tile_pool(name="psx", bufs=2, space="PSUM") as ppx,
            tc.tile_pool(name="psa", bufs=2, space="PSUM") as ppa,
        ):
            idt = cpool.tile([128, 128], f32)
            nc.sync.dma_start(idt[:], ident[:])
            # weight halves: wt[k][h] = MkT[h*128:(h+1)*128, :]  (fp16)
            wt = []
            for k, mt in enumerate((m0t, m1t)):
                row = []
                for h in range(2):
                    w = cpool.tile([128, OUT_DIM], f16, tag=f"w{k}{h}")
                    nc.sync.dma_start(w[:], mt[h * 128:(h + 1) * 128, :])
                    row.append(w)
                wt.append(row)

            tok0 = 0
            for chunk_tok in sched:
                blocks = chunk_tok // 128
                st = min(512, chunk_tok)
                subs = chunk_tok // st
                sblocks = st // 128

                xv = xs[tok0:tok0 + chunk_tok, :].rearrange(
                    "(a p) i -> p a i", p=128
                )
                # tiles are allocated at max-chunk size (tags must keep a
                # constant size across pool rotations); smaller ramp chunks
                # use leading slices.
                xt_f = pool.tile([128, MAXB * IN_DIM], f32, tag="xt")
                xt = xt_f[:, :blocks * IN_DIM]
                nc.sync.dma_start(
                    xt.rearrange("p (a i) -> p a i", i=IN_DIM), xv
                )
                # t16 packed [128, 2*chunk]: col = h*chunk + tok
                t16_f = pool.tile([128, 2 * MAXC], f16, tag="t16")
                t16 = t16_f[:, :2 * chunk_tok]
                t16v = t16.rearrange("p (h t) -> p h t", h=2)
                for s in range(subs):
                    xtp_f = ppx.tile([128, 2 * 512], f32, tag="xtp")
                    xtp = xtp_f[:, :2 * st]
                    for al in range(sblocks):
                        a = s * sblocks + al
                        for h in range(2):
                            nc.tensor.transpose(
                                xtp[:, h * st + al * 128:
                                    h * st + (al + 1) * 128],
                                xt[:, a * IN_DIM + h * 128:
                                   a * IN_DIM + (h + 1) * 128],
                                idt[:],
                            )
                    # tanh fused with PSUM->SBUF, downcast to fp16
                    nc.scalar.activation(
                        t16v[:, :, s * st:(s + 1) * st],
                        xtp.rearrange("p (h t) -> p h t", h=2),
                        Tanh,
                    )

                acc_f = ppa.tile([128, MAXB * OUT_DIM], f32, tag="acc")
                acc = acc_f[:, :blocks * OUT_DIM]
                es = []
                for k in range(2):
                    r_f = pool.tile([128, 2 * MAXC], f16, tag="r")
                    r = r_f[:, :2 * chunk_tok]
                    nc.vector._custom_dve(
                        KAN_CUBE, out=r, in0=t16,
                        s0=S_P, s1=C1P, imm2=C2S[k],
                    )
                    e_f = pool.tile([128, 2 * MAXC], f16, tag=f"e{k}")
                    e = e_f[:, :2 * chunk_tok]
                    nc.vector._custom_dve(
                        KAN_CUBE_ACC, out=e, in0=t16, in1=r,
                        s0=S_Q, s1=C1Q, imm2=C2S[k],
                    )
                    es.append(e)
                # each PSUM region's start..stop accumulation group must be
                # contiguous in PE program order: emit all 4 matmuls per
                # 128-token block together.
                for a in range(blocks):
                    for j in range(4):
                        k, h = j // 2, j % 2
                        nc.tensor.matmul(
                            acc[:, a * OUT_DIM:(a + 1) * OUT_DIM],
                            es[k][:, h * chunk_tok + a * 128:
                                  h * chunk_tok + (a + 1) * 128],
                            wt[k][h][:],
                            start=(j == 0),
                            stop=(j == 3),
                        )
                os_f = pool.tile([128, MAXB * OUT_DIM], f16, tag="os")
                os_t = os_f[:, :blocks * OUT_DIM]
                nc.scalar.copy(os_t, acc)
                ov = out[tok0:tok0 + chunk_tok, :].rearrange(
                    "(a p) o -> p a o", p=128
                )
                nc.sync.dma_start(
                    ov, os_t.rearrange("p (a o) -> p a o", o=OUT_DIM)
                )
                tok0 += chunk_tok

    nc.compile()
    return nc


def _get_nc():
    if "nc" not in _CACHE:
        _CACHE["nc"] = _build_bass()
    return _CACHE["nc"]


def kernel(x, inner_coeffs, outer_coeffs):
    from concourse import bass_utils

    x = np.asarray(x, dtype=np.float32)
    inner = np.asarray(inner_coeffs, dtype=np.float32)
    outer = np.asarray(outer_coeffs, dtype=np.float32)

    B, S, I = x.shape
    xf = np.ascontiguousarray(x.reshape(B * S, I))

    # MkT[i,o] = W[o,i] * c_k[i] / 6  (fp16 on device)
    m0 = np.ascontiguousarray((outer.T * inner[:, 0:1]) / 6.0).astype(np.float16)
    m1 = np.ascontiguousarray((outer.T * inner[:, 1:2]) / 6.0).astype(np.float16)
    ident = np.eye(128, dtype=np.float32)

    nc = _get_nc()
    in_maps = []
    for i in range(N_CORES):
        in_maps.append({
            "xs": xf[i * TOK_PER_CORE:(i + 1) * TOK_PER_CORE],
            "m0t": m0, "m1t": m1, "ident": ident,
        })
    res = bass_utils.run_bass_kernel_spmd(nc, in_maps, list(range(N_CORES)))
    outs = [res.results[i]["out"] for i in range(N_CORES)]
    full = np.concatenate(outs, axis=0).astype(np.float32).reshape(B, S, OUT_DIM)
    return full
